# revision 34
# baseline (speedup 1.0000x reference)
"""Causal self-attention (B=2, S=4096, D=768, H=12) on 8 trn2 NeuronCores.

Sharding: core c -> batch c//4, head-group c%4 (3 heads of 12).
Each core: qkv projection for its heads, causal flash attention in
scores-transposed layout, partial output projection over its heads'
slice of W_out. Host sums the 4 partials per batch (tensor-parallel
unshard).
"""
import json
import sys

sys.path.insert(0, "/opt/trn_rl_repo")

import numpy as np

import concourse.bass as bass
import concourse.mybir as mybir
import concourse.tile as tile
from concourse import bass2jax
from concourse import masks as cmasks
from concourse.bass_utils import run_bass_kernel_spmd

# ---------------------------------------------------------------- BIR patch
# The bundled walrus rejects instructions carrying >1 semaphore wait
# ("Too many sync wait commands" on Tile's kernel-tail Drain). Split
# excess waits onto wait-only EventSemaphore ops inserted just before.
MAX_WAITS = 1


def _split_excess_waits(bir_json: bytes) -> bytes:
    m = json.loads(bir_json)
    n_new = 0
    for fn in m["functions"]:
        for bb in fn["blocks"]:
            new_insts = []
            for inst in bb["instructions"]:
                si = inst.get("sync_info")
                waits = (si or {}).get("on_wait") or []
                if len(waits) > MAX_WAITS:
                    extra = waits[:-MAX_WAITS]
                    inst["sync_info"]["on_wait"] = waits[-MAX_WAITS:]
                    for k in range(0, len(extra), MAX_WAITS):
                        n_new += 1
                        new_insts.append({
                            "debug": inst.get("debug"),
                            "engine": inst["engine"],
                            "ins": [],
                            "outs": [],
                            "name": f"waitsplit-{n_new}",
                            "opcode": "EventSemaphore",
                            "sync_info": {
                                "on_update": [],
                                "on_wait": extra[k:k + MAX_WAITS],
                            },
                        })
                new_insts.append(inst)
            bb["instructions"] = new_insts
    return json.dumps(m).encode()


if not getattr(bass2jax, "_waitsplit_patched", False):
    _orig_compile = bass2jax.compile_bir_kernel

    def _patched_compile(bir_json, tmpdir, neff_name="file.neff"):
        return _orig_compile(_split_excess_waits(bir_json), tmpdir, neff_name)

    bass2jax.compile_bir_kernel = _patched_compile
    bass2jax._waitsplit_patched = True

# ---------------------------------------------------------------- constants
D_MODEL = 768
N_HEAD = 12
D_HEAD = 64
N_CORES = 8
HC = 3           # heads per core
HD = HC * D_HEAD  # 192: local head-dim total
SQ = 512         # query chunk (matmul moving dim)
SK = 128         # key block (scores partition dim)

F32 = mybir.dt.float32
F32R = mybir.dt.float32r
AF = mybir.ActivationFunctionType


def build_program(S=4096, repeat=1, phases=('s1', 'attn', 'out'),
                  psa=2, pss=2, psy=2, mask_eng='vector'):
    NQ = S // SQ
    NK = S // SK
    D = D_MODEL

    nc = bass.Bass("TRN2", target_bir_lowering=False, debug=False, num_devices=1)

    xt = nc.dram_tensor("xt", [D, S], F32R, kind="ExternalInput").ap()
    wq = nc.dram_tensor("wq", [D, HD], F32R, kind="ExternalInput").ap()
    wk = nc.dram_tensor("wk", [D, HD], F32R, kind="ExternalInput").ap()
    wv = nc.dram_tensor("wv", [D, HD], F32R, kind="ExternalInput").ap()
    bq = nc.dram_tensor("bq", [HD, 1], F32, kind="ExternalInput").ap()
    bk = nc.dram_tensor("bk", [HD, 1], F32, kind="ExternalInput").ap()
    bv = nc.dram_tensor("bv", [HD, 1], F32, kind="ExternalInput").ap()
    wo = nc.dram_tensor("wo", [HD + 1, D], F32R, kind="ExternalInput").ap()
    out = nc.dram_tensor("out", [S, D], F32, kind="ExternalOutput").ap()

    KD = D // 128  # 6 contraction chunks for the input projection

    with tile.TileContext(nc) as tc:
        with (
            tc.tile_pool(name="const", bufs=1) as constp,
            tc.tile_pool(name="persist", bufs=1) as persist,
            tc.tile_pool(name="xt", bufs=2) as xtp,
            tc.tile_pool(name="qt", bufs=2) as qtp,
            tc.tile_pool(name="pblk", bufs=4) as pp,
            tc.tile_pool(name="osb", bufs=2) as outp,
            tc.tile_pool(name="norm", bufs=2) as normp,
            tc.tile_pool(name="psA", bufs=psa, space="PSUM") as psA,
            tc.tile_pool(name="psS", bufs=pss, space="PSUM") as psS,
            tc.tile_pool(name="psY", bufs=psy, space="PSUM") as psY,
        ):
          from contextlib import nullcontext
          with (tc.For_i(0, repeat, 1) if repeat > 1 else nullcontext()):
            # ---- constants
            ident = constp.tile([128, 128], F32)
            cmasks.make_identity(nc, ident[:])
            ones1 = constp.tile([1, 64], F32R)
            nc.gpsimd.memset(ones1[:].bitcast(F32), 1.0)

            masks4 = constp.tile([128, 4 * SQ], F32R)
            nc.gpsimd.memset(masks4[:].bitcast(F32), 1.0)
            for p in range(4):
                reg = masks4[:, p * SQ:(p + 1) * SQ].bitcast(F32)
                nc.gpsimd.affine_select(
                    out=reg, in_=reg,
                    compare_op=mybir.AluOpType.is_ge, fill=0.0,
                    base=-128 * p, channel_multiplier=-1, pattern=[[1, SQ]],
                )

            # ---- weights to SBUF
            wq_sb = constp.tile([128, KD * HD], F32R)
            wk_sb = constp.tile([128, KD * HD], F32R)
            wv_sb = constp.tile([128, KD * HD], F32R)
            for w_sb, w in ((wq_sb, wq), (wk_sb, wk), (wv_sb, wv)):
                for ki in range(KD):
                    nc.sync.dma_start(
                        w_sb[:, ki * HD:(ki + 1) * HD],
                        w[ki * 128:(ki + 1) * 128, :])

            # wo as 3 per-head chunks (base partition 0); chunk 2 row 64 = bias
            wo_sb = constp.tile([65, 3 * D], F32R)
            for h in range(HC):
                nc.sync.dma_start(wo_sb[0:64, h * D:(h + 1) * D],
                                  wo[h * 64:(h + 1) * 64, :])
            nc.sync.dma_start(wo_sb[64:65, 2 * D:3 * D], wo[HD:HD + 1, :])

            bias_sb = constp.tile([128, 6], F32)  # q0 q1 k0 k1 v0 v1
            for ci, b in enumerate((bq, bk, bv)):
                nc.sync.dma_start(bias_sb[0:128, 2 * ci:2 * ci + 1], b[0:128, :])
                nc.sync.dma_start(
                    bias_sb[0:HD - 128, 2 * ci + 1:2 * ci + 2], b[128:HD, :])

            # ---- persistent activations
            # kT: [192, S] stored as [128, 2S]: dims 0:128 at cols 0:S,
            # dims 128:192 at cols S:2S (partitions 0:64).
            kT = persist.tile([128, 2 * S], F32R)
            # y per head at base partition 0; head 2 carries a ones row
            # (row 64) so the output projection picks up the bias row of wo.
            y_sb = [persist.tile([65 if h == 2 else 64, S], F32R,
                                 tag=f"y{h}", name=f"y_sb{h}")
                    for h in range(HC)]
            nc.gpsimd.memset(y_sb[2][64:65, :].bitcast(F32), 1.0)
            # v (natural) per head: NK blocks [128, 65], col 64 = ones.
            v_sb = [persist.tile([128, NK * 65], F32R, tag=f"v{h}",
                                 name=f"v_sb{h}")
                    for h in range(HC)]
            for h in range(HC):
                vv = v_sb[h].rearrange("p (n c) -> p n c", c=65)
                nc.gpsimd.memset(vv[:, :, 64].bitcast(F32), 1.0)

            # phase-isolation support: seed tiles a skipped phase would write
            if 'attn' in phases and 's1' not in phases:
                nc.vector.memset(kT[:].bitcast(F32), 0.01)
                for h in range(HC):
                    nc.vector.memset(v_sb[h][:].bitcast(F32), 0.01)
            if 'out' in phases and 'attn' not in phases:
                for h in range(HC):
                    nc.vector.memset(y_sb[h][:].bitcast(F32), 0.01)

            # ========= stage 1+2 fused per query chunk j: xT load,
            # projections, then causal attention for the 3 heads.
            for j in range(NQ):
                qTj = qtp.tile([128, 2 * SQ], F32R, tag="qTj", name=f"qT_{j}")
                if 's1' not in phases:
                    if 'attn' in phases:
                        nc.vector.memset(qTj[:].bitcast(F32), 0.01)
                    xts = None
                else:
                  xts = []
                  for ki in range(KD):
                    t = xtp.tile([128, SQ], F32R, tag=f"xt{ki}",
                                 name=f"xt_{j}_{ki}")
                    nc.sync.dma_start(
                        t[:], xt[ki * 128:(ki + 1) * 128, j * SQ:(j + 1) * SQ])
                    xts.append(t)

                # projections: out rows = qkv dims (2 chunks: 128 + 64)
                for mci, (m0, mlen) in enumerate(((0, 128), (128, 64))) if xts else ():
                    pq = psA.tile([128, SQ], F32, tag="psA")
                    for ki in range(KD):
                        nc.tensor.matmul(
                            pq[0:mlen, :],
                            wq_sb[:, ki * HD + m0:ki * HD + m0 + mlen],
                            xts[ki][:],
                            start=(ki == 0), stop=(ki == KD - 1))
                    nc.scalar.activation(
                        qTj[0:mlen, mci * SQ:(mci + 1) * SQ],
                        pq[0:mlen, :], AF.Identity,
                        bias=bias_sb[0:mlen, mci:mci + 1])
                for mci, (m0, mlen) in enumerate(((0, 128), (128, 64))) if xts else ():
                    pk = psA.tile([128, SQ], F32, tag="psA")
                    for ki in range(KD):
                        nc.tensor.matmul(
                            pk[0:mlen, :],
                            wk_sb[:, ki * HD + m0:ki * HD + m0 + mlen],
                            xts[ki][:],
                            start=(ki == 0), stop=(ki == KD - 1))
                    nc.scalar.activation(
                        kT[0:mlen, mci * S + j * SQ:mci * S + (j + 1) * SQ],
                        pk[0:mlen, :], AF.Identity,
                        bias=bias_sb[0:mlen, 2 + mci:3 + mci])

                # v: project transposed, then PE-transpose per head into
                # natural [Sk,64] blocks (col 64 stays ones)
                for mci, (m0, mlen) in enumerate(((0, 128), (128, 64))) if xts else ():
                    pv = psA.tile([128, SQ], F32, tag="psA")
                    for ki in range(KD):
                        nc.tensor.matmul(
                            pv[0:mlen, :],
                            wv_sb[:, ki * HD + m0:ki * HD + m0 + mlen],
                            xts[ki][:],
                            start=(ki == 0), stop=(ki == KD - 1))
                    vt_sb = xtp.tile([128, SQ], F32, tag="vt")
                    nc.scalar.activation(
                        vt_sb[0:mlen, :], pv[0:mlen, :], AF.Identity,
                        bias=bias_sb[0:mlen, 4 + mci:5 + mci])
                    # heads covered by this m-chunk: mci==0 -> h0 (rows 0:64),
                    # h1 (rows 64:128); mci==1 -> h2 (rows 0:64)
                    heads = ((0, 0), (1, 64)) if mci == 0 else ((2, 0),)
                    for (h, r0) in heads:
                        for si in range(SQ // 128):
                            pt = psA.tile([128, 128], F32, tag="psA")
                            nc.tensor.transpose(
                                pt[0:128, 0:64],
                                vt_sb[r0:r0 + 64, si * 128:(si + 1) * 128],
                                ident[r0:r0 + 64, r0:r0 + 64])
                            blk = j * (SQ // 128) + si
                            nc.vector.tensor_copy(
                                v_sb[h][:, blk * 65:blk * 65 + 64],
                                pt[:, 0:64])

                # ---- causal attention for this query chunk
                # key blocks processed in pairs: one exp covers [128, 1024]
                # across two PSUM banks, amortizing the ACT fixed overhead.
                ilim = 4 * j + 4  # causal: key blocks 0 .. 4j+3
                npair = ilim // 2
                for h in range(HC) if 'attn' in phases else ():
                    qh = qTj[(h % 2) * 64:(h % 2) * 64 + 64,
                             (h // 2) * SQ:(h // 2) * SQ + SQ]
                    py = psY.tile([65, SQ], F32, tag="psY")

                    def qk_pair(m):
                        ps = psS.tile([128, 2 * SQ], F32, tag="psS",
                                      name=f"ps_{j}_{h}_{m}")
                        for t, i in ((0, 2 * m), (1, 2 * m + 1)):
                            kh = kT[(h % 2) * 64:(h % 2) * 64 + 64,
                                    (h // 2) * S + i * SK:(h // 2) * S + (i + 1) * SK]
                            nc.tensor.matmul(ps[:, t * SQ:(t + 1) * SQ], kh, qh,
                                             start=True, stop=True)
                        P = pp.tile([128, 2 * SQ], F32R, tag="P",
                                    name=f"P_{j}_{h}_{m}")
                        nc.scalar.activation(P[:], ps[:], AF.Exp)
                        mrel = m - 2 * j
                        if mrel >= 0:  # the two diagonal pairs get masked
                            getattr(nc, mask_eng).tensor_mul(
                                P[:], P[:],
                                masks4[:, mrel * 2 * SQ:(mrel + 1) * 2 * SQ])
                        return P

                    def pv_pair(m, P):
                        for t, i in ((0, 2 * m), (1, 2 * m + 1)):
                            nc.tensor.matmul(
                                py[:], v_sb[h][:, i * 65:(i + 1) * 65],
                                P[:, t * SQ:(t + 1) * SQ],
                                start=(i == 0), stop=(i == ilim - 1))

# simple order: scheduler pipelines by readiness
                    for m in range(npair):
                        pv_pair(m, qk_pair(m))
                    # normalize rows 0:64 by row 64 (sum of exp)
                    recip = normp.tile([1, SQ], F32, tag="recip")
                    nc.vector.reciprocal(recip[:], py[64:65, :])
                    recip_r = normp.tile([1, SQ], F32R, tag="recip_r")
                    nc.vector.tensor_copy(recip_r[:], recip[:])
                    # broadcast recip across 64 partitions via a K=1 matmul
                    pbc = psS.tile([128, SQ], F32, tag="psS")
                    nc.tensor.matmul(pbc[0:64, :], ones1[:], recip_r[:],
                                     start=True, stop=True)
                    bc = normp.tile([64, SQ], F32, tag="bc")
                    nc.vector.tensor_copy(bc[:], pbc[0:64, :])
                    nc.vector.tensor_mul(
                        y_sb[h][0:64, j * SQ:(j + 1) * SQ], py[0:64, :], bc[:])

            # ================= stage 3: partial output projection
            for sq in range(S // 128) if 'out' in phases else ():
                osb = outp.tile([128, D], F32, tag="osb")
                for (n0, nlen) in ((0, 512), (512, 256)):
                    po = psA.tile([128, SQ], F32, tag="psA")
                    for h in range(HC):
                        kk = 65 if h == 2 else 64
                        nc.tensor.matmul(
                            po[:, 0:nlen],
                            y_sb[h][0:kk, sq * 128:(sq + 1) * 128],
                            wo_sb[0:kk, h * D + n0:h * D + n0 + nlen],
                            start=(h == 0), stop=(h == HC - 1))
                    nc.vector.tensor_copy(osb[:, n0:n0 + nlen], po[:, 0:nlen])
                nc.sync.dma_start(out[sq * 128:(sq + 1) * 128, :], osb[:])

    return nc


_prog_cache = {}


def get_program(S=4096):
    if S not in _prog_cache:
        _prog_cache[S] = build_program(S)
    return _prog_cache[S]


# ------------------------------------------------------- cached PJRT runner
# Mirrors bass2jax.run_bass_via_pjrt but jits once and keeps the zero
# output buffers device-resident (no donation: the kernel writes every
# output element, so uninitialized result buffers are fine).
class Runner:
    def __init__(self, nc, n_cores=N_CORES):
        import jax
        from jax.experimental.shard_map import shard_map
        from jax.sharding import Mesh, NamedSharding, PartitionSpec

        bass2jax.install_neuronx_cc_hook()
        assert nc.dbg_addr is None
        partition_name = (nc.partition_id_tensor.name
                          if nc.partition_id_tensor else None)
        in_names, out_names, out_avals, zero_outs = [], [], [], []
        for alloc in nc.m.functions[0].allocations:
            if not isinstance(alloc, mybir.MemoryLocationSet):
                continue
            name = alloc.memorylocations[0].name
            if alloc.kind == "ExternalInput":
                if name != partition_name:
                    in_names.append(name)
            elif alloc.kind == "ExternalOutput":
                shape = tuple(alloc.tensor_shape)
                dtype = mybir.dt.np(alloc.dtype)
                out_names.append(name)
                out_avals.append(jax.core.ShapedArray(shape, dtype))
                zero_outs.append(np.zeros(shape, dtype))
        self.in_names = list(in_names)
        self.out_names = list(out_names)
        self.out_avals = out_avals
        all_in = tuple(in_names + out_names +
                       ([partition_name] if partition_name else []))

        def _body(*args):
            operands = list(args)
            if partition_name:
                operands.append(bass2jax.partition_id_tensor())
            outs = bass2jax._bass_exec_p.bind(
                *operands,
                out_avals=tuple(out_avals),
                in_names=all_in,
                out_names=tuple(out_names),
                lowering_input_output_aliases=(),
                sim_require_finite=True,
                sim_require_nnan=True,
                nc=nc,
            )
            return tuple(outs)

        devices = jax.devices()[:n_cores]
        assert len(devices) == n_cores
        self.mesh = Mesh(np.asarray(devices), ("core",))
        self.sharding = NamedSharding(self.mesh, PartitionSpec("core"))
        nin = len(self.in_names) + len(out_names)
        self.fn = jax.jit(
            shard_map(
                _body, mesh=self.mesh,
                in_specs=(PartitionSpec("core"),) * nin,
                out_specs=(PartitionSpec("core"),) * len(out_names),
                check_rep=False),
            keep_unused=True)
        self.zeros_dev = [
            jax.device_put(
                np.zeros((n_cores * z.shape[0], *z.shape[1:]), z.dtype),
                self.sharding)
            for z in zero_outs]
        self.n_cores = n_cores

    def concat_inputs(self, in_maps):
        return [np.concatenate([np.asarray(in_maps[c][n])
                                for c in range(self.n_cores)], axis=0)
                for n in self.in_names]

    def put(self, concat_in):
        import jax
        return [jax.device_put(a, self.sharding) for a in concat_in]

    def __call__(self, dev_in):
        return self.fn(*dev_in, *self.zeros_dev)

    def run(self, in_maps):
        outs = self(self.put(self.concat_inputs(in_maps)))
        return [
            {name: np.asarray(outs[i]).reshape(
                self.n_cores, *self.out_avals[i].shape)[c]
             for i, name in enumerate(self.out_names)}
            for c in range(self.n_cores)]


_runner_cache = {}


def get_runner(S=4096):
    if S not in _runner_cache:
        _runner_cache[S] = Runner(get_program(S))
    return _runner_cache[S]


def make_in_maps(x, W_in, b_in, W_out, b_out):
    B, S, D = x.shape
    in_maps = []
    for c in range(N_CORES):
        b = c // 4
        g = c % 4
        cs = slice(HD * g, HD * g + HD)
        qcols = np.arange(D)[cs]
        wq = (W_in[:, 0 * D:1 * D][:, cs] / 8.0).astype(np.float32)
        wk = W_in[:, 1 * D:2 * D][:, cs].astype(np.float32)
        wv = W_in[:, 2 * D:3 * D][:, cs].astype(np.float32)
        bq = (b_in[0 * D:1 * D][cs] / 8.0).astype(np.float32).reshape(HD, 1)
        bk = b_in[1 * D:2 * D][cs].astype(np.float32).reshape(HD, 1)
        bv = b_in[2 * D:3 * D][cs].astype(np.float32).reshape(HD, 1)
        bo = b_out if g == 0 else np.zeros_like(b_out)
        wo = np.concatenate(
            [W_out[cs, :], bo[None, :]], axis=0).astype(np.float32)
        in_maps.append({
            "xt": np.ascontiguousarray(x[b].T).astype(np.float32),
            "wq": np.ascontiguousarray(wq), "wk": np.ascontiguousarray(wk),
            "wv": np.ascontiguousarray(wv),
            "bq": bq, "bk": bk, "bv": bv,
            "wo": np.ascontiguousarray(wo),
        })
    return in_maps


def kernel(x, W_in, b_in, W_out, b_out):
    x = np.asarray(x)
    B, S, D = x.shape
    runner = get_runner(S)
    in_maps = make_in_maps(np.asarray(x), np.asarray(W_in), np.asarray(b_in),
                           np.asarray(W_out), np.asarray(b_out))
    results = runner.run(in_maps)
    out = np.zeros((B, S, D), dtype=np.float32)
    for c in range(N_CORES):
        out[c // 4] += results[c]["out"]
    return out
